# revision 7
# baseline (speedup 1.0000x reference)
"""Trainium2 Bass kernel for nn_Attention2 (B=4, N=4096, W=1024, H=16, A=64).

Sharding: 8 cores = batch(4) x head-half(2). Each core computes the partial
output sum over its 8 heads for one batch; the host adds the two half-sums.

Math (per batch b, head h):
    cross_e = exp(x@k1 + p1);  e = exp(x@(k2-k3) - p2)   [= diag/(extra*p2e)]
    C[n] = cumsum(cross_e);  r = 1/(C + e)
    Shat[t] = (cross_e[t-1]/cross_e[t]) * Shat[t-1] + v[t]      (v = x@vw)
    out = Shat*(cross_e*r) + v*(e*r);  y = sum_h out @ owT
p1/p2 (sums of 64 near-linear sinusoids, |arg|<=0.1) are expanded as cubic
polynomials in n and folded into the k-projection matmul via 4 extra
contraction rows of [1, n, n^2, n^3].
"""

import numpy as np

import concourse.bacc as bacc
import concourse.mybir as mybir
import concourse.tile as tile
from concourse.bass_utils import run_bass_kernel_spmd

F32 = mybir.dt.float32
F32R = mybir.dt.float32r
AF = mybir.ActivationFunctionType
OP = mybir.AluOpType

B, N, W, H, A, P = 4, 4096, 1024, 16, 64, 64
HL = 8            # heads per core
NPAIR = 4         # head pairs per core
CHUNK = 512
NCHUNK = N // CHUNK          # 8
KB = W // 128                # 8 x-K-blocks
NBLK = CHUNK // 128          # n-blocks per chunk for stage-3

_NC_CACHE = {}


def _build():
    if "nc" in _NC_CACHE:
        return _NC_CACHE["nc"]
    nc = bacc.Bacc("TRN2")

    xtb = nc.dram_tensor("xtb", [W + 4, N], F32R, kind="ExternalInput")
    kpack = nc.dram_tensor("kpack", [W + 4, 72], F32R, kind="ExternalInput")
    vwp = nc.dram_tensor("vwp", [NPAIR, W, 128], F32R, kind="ExternalInput")
    owtp = nc.dram_tensor("owtp", [NPAIR, 128, W], F32R, kind="ExternalInput")
    selp = nc.dram_tensor("selp", [NPAIR, HL, 128], F32R, kind="ExternalInput")
    y = nc.dram_tensor("y", [N, W], F32, kind="ExternalOutput")

    with tile.TileContext(nc) as tc:
        with (
            tc.tile_pool(name="const", bufs=1) as const,
            tc.tile_pool(name="xtp", bufs=2) as xtp,
            tc.tile_pool(name="rowp", bufs=2) as rowp,
            tc.tile_pool(name="bigp", bufs=3) as bigp,
            tc.tile_pool(name="innp", bufs=8) as innp,
            tc.tile_pool(name="yp", bufs=2) as yp,
            tc.tile_pool(name="v_ps", bufs=2, space="PSUM") as v_ps,
            tc.tile_pool(name="bc_ps", bufs=1, space="PSUM") as bc_ps,
            tc.tile_pool(name="rows_ps", bufs=1, space="PSUM") as rows_ps,
            tc.tile_pool(name="y_ps", bufs=2, space="PSUM") as y_ps,
        ):
            # ---- resident weights ----
            kp_sb = []
            for kb in range(KB):
                kpt = const.tile([128, 72], F32R, name=f"kp{kb}", tag=f"kp{kb}")
                nc.sync.dma_start(out=kpt, in_=kpack[kb * 128:(kb + 1) * 128, :])
                kp_sb.append(kpt)
            kp4 = const.tile([4, 72], F32R, name="kpbas", tag="kpbas")
            nc.sync.dma_start(out=kp4, in_=kpack[W:W + 4, :])

            vw_sb = []
            for p in range(NPAIR):
                row = []
                for kb in range(KB):
                    t = const.tile([128, 128], F32R, name=f"vw{p}_{kb}", tag=f"vw{p}_{kb}")
                    nc.sync.dma_start(
                        out=t, in_=vwp[p, kb * 128:(kb + 1) * 128, :])
                    row.append(t)
                vw_sb.append(row)

            owt_sb = []
            for p in range(NPAIR):
                t = const.tile([128, W], F32R, name=f"owt{p}", tag=f"owt{p}")
                nc.sync.dma_start(out=t, in_=owtp[p, :, :])
                owt_sb.append(t)

            sel_sb = []
            for p in range(NPAIR):
                t = const.tile([HL, 128], F32R, name=f"sel{p}", tag=f"sel{p}")
                nc.sync.dma_start(out=t, in_=selp[p, :, :])
                sel_sb.append(t)

            ones8 = const.tile([HL, CHUNK], F32)
            nc.vector.memset(ones8, 1.0)

            s_prev = [None] * NPAIR     # Shat carry tiles per pair
            c_prev = None               # C-scan carry tile
            c_prev_t = None             # previous chunk's c tile

            for ci in range(NCHUNK):
                c0 = ci * CHUNK
                # ---- load xT chunk ----
                xt = []
                for kb in range(KB):
                    t = xtp.tile([128, CHUNK], F32R, name=f"xt{kb}", tag=f"xt{kb}")
                    nc.sync.dma_start(
                        out=t, in_=xtb[kb * 128:(kb + 1) * 128, c0:c0 + CHUNK])
                    xt.append(t)
                bas = xtp.tile([4, CHUNK], F32R, tag="bas")
                nc.sync.dma_start(out=bas, in_=xtb[W:W + 4, c0:c0 + CHUNK])

                # ---- row projections: [24, CHUNK] ----
                rows = rows_ps.tile([72, CHUNK], F32, tag="rows")
                for kb in range(KB):
                    nc.tensor.matmul(rows, lhsT=kp_sb[kb], rhs=xt[kb],
                                     start=(kb == 0), stop=False)
                nc.tensor.matmul(rows, lhsT=kp4, rhs=bas,
                                 start=False, stop=True)

                # exp the three row groups into base-0 SBUF tiles
                c_t = rowp.tile([HL, CHUNK], F32, tag="c_t")
                nc.scalar.activation(c_t, rows[0:8, :], AF.Exp)
                rcpc_t = rowp.tile([HL, CHUNK], F32, tag="rcpc_t")
                nc.scalar.activation(rcpc_t, rows[32:40, :], AF.Exp)
                e_t = rowp.tile([HL, CHUNK], F32, tag="e_t")
                nc.scalar.activation(e_t, rows[64:72, :], AF.Exp)

                # ratio[t] = c[t-1] * rcpc[t]
                ratio = rowp.tile([HL, CHUNK], F32R, tag="ratio")
                nc.vector.tensor_mul(ratio[:, 1:CHUNK], c_t[:, 0:CHUNK - 1],
                                     rcpc_t[:, 1:CHUNK])
                if c_prev_t is None:
                    # any finite value works: initial state is 0
                    nc.vector.tensor_copy(ratio[:, 0:1], ones8[:, 0:1])
                else:
                    nc.vector.tensor_mul(ratio[:, 0:1],
                                         c_prev_t[:, CHUNK - 1:CHUNK],
                                         rcpc_t[:, 0:1])
                c_prev_t = c_t
                # C = cumsum(c) chunk-chained
                c_ch = rowp.tile([HL, CHUNK], F32, tag="c_ch")
                nc.vector.tensor_tensor_scan(
                    c_ch, data0=ones8, data1=c_t,
                    initial=(0.0 if c_prev is None else c_prev[:, CHUNK - 1:CHUNK]),
                    op0=OP.mult, op1=OP.add)
                c_prev = c_ch
                # den = C + e ; rden = 1/den
                den = rowp.tile([HL, CHUNK], F32, tag="den")
                nc.vector.tensor_add(den, c_ch, e_t)
                rden = rowp.tile([HL, CHUNK], F32, tag="rden")
                nc.vector.reciprocal_approx_fast(out=rden, in_=den)
                # r2 = c * rden ; q2 = e * rden
                r2 = rowp.tile([HL, CHUNK], F32R, tag="r2")
                nc.vector.tensor_mul(r2, c_t, rden)
                q2 = rowp.tile([HL, CHUNK], F32R, tag="q2")
                nc.vector.tensor_mul(q2, e_t, rden)

                inner = []
                for p in range(NPAIR):
                    # ---- values pair matmul ----
                    vps = v_ps.tile([128, CHUNK], F32, tag="v")
                    for kb in range(KB):
                        nc.tensor.matmul(vps, lhsT=vw_sb[p][kb], rhs=xt[kb],
                                         start=(kb == 0), stop=(kb == KB - 1))
                    v_sb = bigp.tile([128, CHUNK], F32, tag="v_sb")
                    nc.scalar.copy(v_sb, vps)

                    # ---- broadcast row scalars across partitions ----
                    ratio_rep = bc_ps.tile([128, CHUNK], F32, tag="ratio_rep")
                    nc.tensor.matmul(ratio_rep, lhsT=sel_sb[p], rhs=ratio,
                                     start=True, stop=True)
                    r2_rep = bc_ps.tile([128, CHUNK], F32, tag="r2_rep")
                    nc.tensor.matmul(r2_rep, lhsT=sel_sb[p], rhs=r2,
                                     start=True, stop=True)
                    q2_rep = bc_ps.tile([128, CHUNK], F32, tag="q2_rep")
                    nc.tensor.matmul(q2_rep, lhsT=sel_sb[p], rhs=q2,
                                     start=True, stop=True)

                    # ---- Shat scan ----
                    s_sb = bigp.tile([128, CHUNK], F32, tag="s_sb", bufs=8)
                    nc.vector.tensor_tensor_scan(
                        s_sb, data0=ratio_rep, data1=v_sb,
                        initial=(0.0 if s_prev[p] is None
                                 else s_prev[p][:, CHUNK - 1:CHUNK]),
                        op0=OP.mult, op1=OP.add)
                    s_prev[p] = s_sb

                    # t1 = Shat * r2_rep ; t2 = v * q2_rep (in place)
                    t1 = bigp.tile([128, CHUNK], F32, tag="t1")
                    nc.vector.tensor_mul(t1, s_sb, r2_rep)
                    nc.vector.tensor_mul(v_sb, v_sb, q2_rep)
                    inn = innp.tile([128, CHUNK], F32R, name="inner", tag="inner")
                    nc.gpsimd.tensor_add(inn, t1, v_sb)
                    inner.append(inn)

                # ---- stage 3: y[n, w] ----
                for nb in range(NBLK):
                    y_sb = yp.tile([128, W], F32, tag="y_sb")
                    for wh in range(2):
                        yps = y_ps.tile([128, 512], F32, tag="y")
                        for p in range(NPAIR):
                            nc.tensor.matmul(
                                yps,
                                lhsT=inner[p][:, nb * 128:(nb + 1) * 128],
                                rhs=owt_sb[p][:, wh * 512:(wh + 1) * 512],
                                start=(p == 0), stop=(p == NPAIR - 1))
                        nc.scalar.copy(y_sb[:, wh * 512:(wh + 1) * 512], yps)
                    nc.sync.dma_start(
                        out=y[c0 + nb * 128:c0 + (nb + 1) * 128, :], in_=y_sb)

    nc.finalize()
    _NC_CACHE["nc"] = nc
    return nc


def _host_prep(x, k1, k2, k3, a1, a2, b1, b2, c, value_weight, output_weight):
    """Build the 8 per-core input maps."""
    x = np.asarray(x, np.float32)
    k1 = np.asarray(k1, np.float32)
    k2 = np.asarray(k2, np.float32)
    k3 = np.asarray(k3, np.float32)
    a1 = np.asarray(a1, np.float64)[..., 0]   # [H, P]
    a2 = np.asarray(a2, np.float64)[..., 0]
    b1 = np.asarray(b1, np.float64)[..., 0]
    b2 = np.asarray(b2, np.float64)[..., 0]
    cc = np.asarray(c, np.float64)[..., 0]
    vw = np.asarray(value_weight, np.float32)   # [H, W, A]
    ow = np.asarray(output_weight, np.float32)  # [H, W, A]

    n = np.linspace(0.0, 1.0, N)
    basis = np.stack([np.ones_like(n), n, n * n, n ** 3]).astype(np.float32)

    def taylor(a, b):
        # coef[k, h] of n^k for sum_p c*sin(a*n+b)
        s, co = np.sin(b), np.cos(b)
        c0 = (cc * s).sum(1)
        c1 = (cc * a * co).sum(1)
        c2 = -(cc * a * a * s).sum(1) / 2.0
        c3 = -(cc * a ** 3 * co).sum(1) / 6.0
        return np.stack([c0, c1, c2, c3])      # [4, H]

    p1c = taylor(a1, b1)
    p2c = taylor(a2, b2)

    xt_by_b = [np.empty((W + 4, N), np.float32) for _ in range(B)]
    for b in range(B):
        xt_by_b[b][:W] = x[b].T
        xt_by_b[b][W:] = basis

    selp = np.zeros((NPAIR, HL, 128), np.float32)
    for p in range(NPAIR):
        selp[p, 2 * p, 0:64] = 1.0
        selp[p, 2 * p + 1, 64:128] = 1.0

    in_maps = []
    for core in range(8):
        b, half = divmod(core, 2)
        hs = slice(half * HL, (half + 1) * HL)
        kpk = np.zeros((W + 4, 72), np.float32)
        kpk[:W, 0:8] = k1[hs].T
        kpk[W:, 0:8] = p1c[:, hs]
        kpk[:W, 32:40] = -k1[hs].T
        kpk[W:, 32:40] = -p1c[:, hs]
        kpk[:W, 64:72] = (k2[hs] - k3[hs]).T
        kpk[W:, 64:72] = -p2c[:, hs]

        vwp = np.empty((NPAIR, W, 128), np.float32)
        owtp = np.empty((NPAIR, 128, W), np.float32)
        for p in range(NPAIR):
            h0 = half * HL + 2 * p
            vwp[p, :, 0:64] = vw[h0]
            vwp[p, :, 64:128] = vw[h0 + 1]
            owtp[p, 0:64, :] = ow[h0].T
            owtp[p, 64:128, :] = ow[h0 + 1].T

        in_maps.append(dict(xtb=xt_by_b[b], kpack=kpk, vwp=vwp, owtp=owtp,
                            selp=selp))
    return in_maps


LAST_RESULT = None


def kernel(**inputs) -> np.ndarray:
    global LAST_RESULT
    in_maps = _host_prep(**inputs)
    nc = _build()
    import os
    trace = bool(int(os.environ.get("KERNEL_TRACE", "0")))
    res = run_bass_kernel_spmd(nc, in_maps, core_ids=list(range(8)),
                               trace=trace)
    LAST_RESULT = res
    out = np.empty((B, N, W), np.float32)
    for b in range(B):
        out[b] = res.results[2 * b]["y"] + res.results[2 * b + 1]["y"]
    return out


# revision 8
# speedup vs baseline: 3.0270x; 3.0270x over previous
"""Trainium2 Bass kernel for nn_Attention2 (B=4, N=4096, W=1024, H=16, A=64).

Sharding: 8 cores = batch(4) x head-half(2). Each core computes the partial
output sum over its 8 heads for one batch; the host adds the two half-sums.

Math (per batch b, head h):
    cross_e = exp(x@k1 + p1);  e = exp(x@(k2-k3) - p2)   [= diag/(extra*p2e)]
    C[n] = cumsum(cross_e);  r = 1/(C + e)
    Shat[t] = (cross_e[t-1]/cross_e[t]) * Shat[t-1] + v[t]      (v = x@vw)
    out = Shat*(cross_e*r) + v*(e*r);  y = sum_h out @ owT
p1/p2 (sums of 64 near-linear sinusoids, |arg|<=0.1) are expanded as cubic
polynomials in n and folded into the k-projection matmul via 4 extra
contraction rows of [1, n, n^2, n^3].
"""

import numpy as np

import concourse.bacc as bacc
import concourse.mybir as mybir
import concourse.tile as tile
from concourse.bass_utils import run_bass_kernel_spmd

F32 = mybir.dt.float32
F32R = mybir.dt.float32r
AF = mybir.ActivationFunctionType
OP = mybir.AluOpType

B, N, W, H, A, P = 4, 4096, 1024, 16, 64, 64
HL = 8            # heads per core
NPAIR = 4         # head pairs per core
CHUNK = 512
NCHUNK = N // CHUNK          # 8
KB = W // 128                # 8 x-K-blocks
NBLK = CHUNK // 128          # n-blocks per chunk for stage-3

_NC_CACHE = {}


def _build(reps=1):
    key = ("nc", reps)
    if key in _NC_CACHE:
        return _NC_CACHE[key]
    nc = bacc.Bacc("TRN2")

    xtb = nc.dram_tensor("xtb", [W + 4, N], F32R, kind="ExternalInput")
    kpack = nc.dram_tensor("kpack", [W + 4, 72], F32R, kind="ExternalInput")
    vwp = nc.dram_tensor("vwp", [NPAIR, W, 128], F32R, kind="ExternalInput")
    owtp = nc.dram_tensor("owtp", [NPAIR, 128, W], F32R, kind="ExternalInput")
    selp = nc.dram_tensor("selp", [NPAIR, HL, 128], F32R, kind="ExternalInput")
    y = nc.dram_tensor("y", [N, W], F32, kind="ExternalOutput")

    with tile.TileContext(nc) as tc:
        with (
            tc.tile_pool(name="const", bufs=1) as const,
            tc.tile_pool(name="xtp", bufs=2) as xtp,
            tc.tile_pool(name="rowp", bufs=2) as rowp,
            tc.tile_pool(name="bigp", bufs=3) as bigp,
            tc.tile_pool(name="innp", bufs=8) as innp,
            tc.tile_pool(name="yp", bufs=2) as yp,
            tc.tile_pool(name="v_ps", bufs=2, space="PSUM") as v_ps,
            tc.tile_pool(name="bc_ps", bufs=1, space="PSUM") as bc_ps,
            tc.tile_pool(name="rows_ps", bufs=1, space="PSUM") as rows_ps,
            tc.tile_pool(name="y_ps", bufs=2, space="PSUM") as y_ps,
        ):
            # ---- resident weights ----
            kp_sb = []
            for kb in range(KB):
                kpt = const.tile([128, 72], F32R, name=f"kp{kb}", tag=f"kp{kb}")
                nc.sync.dma_start(out=kpt, in_=kpack[kb * 128:(kb + 1) * 128, :])
                kp_sb.append(kpt)
            kp4 = const.tile([4, 72], F32R, name="kpbas", tag="kpbas")
            nc.sync.dma_start(out=kp4, in_=kpack[W:W + 4, :])

            vw_sb = []
            for p in range(NPAIR):
                row = []
                for kb in range(KB):
                    t = const.tile([128, 128], F32R, name=f"vw{p}_{kb}", tag=f"vw{p}_{kb}")
                    nc.sync.dma_start(
                        out=t, in_=vwp[p, kb * 128:(kb + 1) * 128, :])
                    row.append(t)
                vw_sb.append(row)

            owt_sb = []
            for p in range(NPAIR):
                t = const.tile([128, W], F32R, name=f"owt{p}", tag=f"owt{p}")
                nc.sync.dma_start(out=t, in_=owtp[p, :, :])
                owt_sb.append(t)

            sel_sb = []
            for p in range(NPAIR):
                t = const.tile([HL, 128], F32R, name=f"sel{p}", tag=f"sel{p}")
                nc.sync.dma_start(out=t, in_=selp[p, :, :])
                sel_sb.append(t)

            ones8 = const.tile([HL, CHUNK], F32)
            nc.vector.memset(ones8, 1.0)

            s_prev = [None] * NPAIR     # Shat carry tiles per pair
            c_prev = None               # C-scan carry tile
            c_prev_t = None             # previous chunk's c tile

            for ci in range(NCHUNK * reps):
                ci = ci % NCHUNK
                c0 = ci * CHUNK
                # ---- load xT chunk ----
                xt = []
                for kb in range(KB):
                    t = xtp.tile([128, CHUNK], F32R, name=f"xt{kb}", tag=f"xt{kb}")
                    nc.sync.dma_start(
                        out=t, in_=xtb[kb * 128:(kb + 1) * 128, c0:c0 + CHUNK])
                    xt.append(t)
                bas = xtp.tile([4, CHUNK], F32R, tag="bas")
                nc.sync.dma_start(out=bas, in_=xtb[W:W + 4, c0:c0 + CHUNK])

                # ---- row projections: [24, CHUNK] ----
                rows = rows_ps.tile([72, CHUNK], F32, tag="rows")
                for kb in range(KB):
                    nc.tensor.matmul(rows, lhsT=kp_sb[kb], rhs=xt[kb],
                                     start=(kb == 0), stop=False)
                nc.tensor.matmul(rows, lhsT=kp4, rhs=bas,
                                 start=False, stop=True)

                # exp the three row groups into base-0 SBUF tiles
                c_t = rowp.tile([HL, CHUNK], F32, tag="c_t")
                nc.scalar.activation(c_t, rows[0:8, :], AF.Exp)
                rcpc_t = rowp.tile([HL, CHUNK], F32, tag="rcpc_t")
                nc.scalar.activation(rcpc_t, rows[32:40, :], AF.Exp)
                e_t = rowp.tile([HL, CHUNK], F32, tag="e_t")
                nc.scalar.activation(e_t, rows[64:72, :], AF.Exp)

                # ratio[t] = c[t-1] * rcpc[t]
                ratio = rowp.tile([HL, CHUNK], F32R, tag="ratio")
                nc.vector.tensor_mul(ratio[:, 1:CHUNK], c_t[:, 0:CHUNK - 1],
                                     rcpc_t[:, 1:CHUNK])
                if c_prev_t is None:
                    # any finite value works: initial state is 0
                    nc.vector.tensor_copy(ratio[:, 0:1], ones8[:, 0:1])
                else:
                    nc.vector.tensor_mul(ratio[:, 0:1],
                                         c_prev_t[:, CHUNK - 1:CHUNK],
                                         rcpc_t[:, 0:1])
                c_prev_t = c_t
                # C = cumsum(c) chunk-chained
                c_ch = rowp.tile([HL, CHUNK], F32, tag="c_ch")
                nc.vector.tensor_tensor_scan(
                    c_ch, data0=ones8, data1=c_t,
                    initial=(0.0 if c_prev is None else c_prev[:, CHUNK - 1:CHUNK]),
                    op0=OP.mult, op1=OP.add)
                c_prev = c_ch
                # den = C + e ; rden = 1/den
                den = rowp.tile([HL, CHUNK], F32, tag="den")
                nc.vector.tensor_add(den, c_ch, e_t)
                rden = rowp.tile([HL, CHUNK], F32, tag="rden")
                nc.vector.reciprocal_approx_fast(out=rden, in_=den)
                # r2 = c * rden ; q2 = e * rden
                r2 = rowp.tile([HL, CHUNK], F32R, tag="r2")
                nc.vector.tensor_mul(r2, c_t, rden)
                q2 = rowp.tile([HL, CHUNK], F32R, tag="q2")
                nc.vector.tensor_mul(q2, e_t, rden)

                inner = []
                for p in range(NPAIR):
                    # ---- values pair matmul ----
                    vps = v_ps.tile([128, CHUNK], F32, tag="v")
                    for kb in range(KB):
                        nc.tensor.matmul(vps, lhsT=vw_sb[p][kb], rhs=xt[kb],
                                         start=(kb == 0), stop=(kb == KB - 1))
                    v_sb = bigp.tile([128, CHUNK], F32, tag="v_sb")
                    nc.scalar.copy(v_sb, vps)

                    # ---- broadcast row scalars across partitions ----
                    ratio_rep = bc_ps.tile([128, CHUNK], F32, tag="ratio_rep")
                    nc.tensor.matmul(ratio_rep, lhsT=sel_sb[p], rhs=ratio,
                                     start=True, stop=True)
                    r2_rep = bc_ps.tile([128, CHUNK], F32, tag="r2_rep")
                    nc.tensor.matmul(r2_rep, lhsT=sel_sb[p], rhs=r2,
                                     start=True, stop=True)
                    q2_rep = bc_ps.tile([128, CHUNK], F32, tag="q2_rep")
                    nc.tensor.matmul(q2_rep, lhsT=sel_sb[p], rhs=q2,
                                     start=True, stop=True)

                    # ---- Shat scan ----
                    s_sb = bigp.tile([128, CHUNK], F32, tag="s_sb", bufs=8)
                    nc.vector.tensor_tensor_scan(
                        s_sb, data0=ratio_rep, data1=v_sb,
                        initial=(0.0 if s_prev[p] is None
                                 else s_prev[p][:, CHUNK - 1:CHUNK]),
                        op0=OP.mult, op1=OP.add)
                    s_prev[p] = s_sb

                    # t1 = Shat * r2_rep ; t2 = v * q2_rep (in place)
                    t1 = bigp.tile([128, CHUNK], F32, tag="t1")
                    nc.vector.tensor_mul(t1, s_sb, r2_rep)
                    nc.vector.tensor_mul(v_sb, v_sb, q2_rep)
                    inn = innp.tile([128, CHUNK], F32R, name="inner", tag="inner")
                    nc.gpsimd.tensor_add(inn, t1, v_sb)
                    inner.append(inn)

                # ---- stage 3: y[n, w] ----
                for nb in range(NBLK):
                    y_sb = yp.tile([128, W], F32, tag="y_sb")
                    for wh in range(2):
                        yps = y_ps.tile([128, 512], F32, tag="y")
                        for p in range(NPAIR):
                            nc.tensor.matmul(
                                yps,
                                lhsT=inner[p][:, nb * 128:(nb + 1) * 128],
                                rhs=owt_sb[p][:, wh * 512:(wh + 1) * 512],
                                start=(p == 0), stop=(p == NPAIR - 1))
                        nc.scalar.copy(y_sb[:, wh * 512:(wh + 1) * 512], yps)
                    nc.sync.dma_start(
                        out=y[c0 + nb * 128:c0 + (nb + 1) * 128, :], in_=y_sb)

    nc.finalize()
    _NC_CACHE[key] = nc
    return nc


def _host_prep(x, k1, k2, k3, a1, a2, b1, b2, c, value_weight, output_weight):
    """Build the 8 per-core input maps."""
    x = np.asarray(x, np.float32)
    k1 = np.asarray(k1, np.float32)
    k2 = np.asarray(k2, np.float32)
    k3 = np.asarray(k3, np.float32)
    a1 = np.asarray(a1, np.float64)[..., 0]   # [H, P]
    a2 = np.asarray(a2, np.float64)[..., 0]
    b1 = np.asarray(b1, np.float64)[..., 0]
    b2 = np.asarray(b2, np.float64)[..., 0]
    cc = np.asarray(c, np.float64)[..., 0]
    vw = np.asarray(value_weight, np.float32)   # [H, W, A]
    ow = np.asarray(output_weight, np.float32)  # [H, W, A]

    n = np.linspace(0.0, 1.0, N)
    basis = np.stack([np.ones_like(n), n, n * n, n ** 3]).astype(np.float32)

    def taylor(a, b):
        # coef[k, h] of n^k for sum_p c*sin(a*n+b)
        s, co = np.sin(b), np.cos(b)
        c0 = (cc * s).sum(1)
        c1 = (cc * a * co).sum(1)
        c2 = -(cc * a * a * s).sum(1) / 2.0
        c3 = -(cc * a ** 3 * co).sum(1) / 6.0
        return np.stack([c0, c1, c2, c3])      # [4, H]

    p1c = taylor(a1, b1)
    p2c = taylor(a2, b2)

    xt_by_b = [np.empty((W + 4, N), np.float32) for _ in range(B)]
    for b in range(B):
        xt_by_b[b][:W] = x[b].T
        xt_by_b[b][W:] = basis

    selp = np.zeros((NPAIR, HL, 128), np.float32)
    for p in range(NPAIR):
        selp[p, 2 * p, 0:64] = 1.0
        selp[p, 2 * p + 1, 64:128] = 1.0

    in_maps = []
    for core in range(8):
        b, half = divmod(core, 2)
        hs = slice(half * HL, (half + 1) * HL)
        kpk = np.zeros((W + 4, 72), np.float32)
        kpk[:W, 0:8] = k1[hs].T
        kpk[W:, 0:8] = p1c[:, hs]
        kpk[:W, 32:40] = -k1[hs].T
        kpk[W:, 32:40] = -p1c[:, hs]
        kpk[:W, 64:72] = (k2[hs] - k3[hs]).T
        kpk[W:, 64:72] = -p2c[:, hs]

        vwp = np.empty((NPAIR, W, 128), np.float32)
        owtp = np.empty((NPAIR, 128, W), np.float32)
        for p in range(NPAIR):
            h0 = half * HL + 2 * p
            vwp[p, :, 0:64] = vw[h0]
            vwp[p, :, 64:128] = vw[h0 + 1]
            owtp[p, 0:64, :] = ow[h0].T
            owtp[p, 64:128, :] = ow[h0 + 1].T

        in_maps.append(dict(xtb=xt_by_b[b], kpack=kpk, vwp=vwp, owtp=owtp,
                            selp=selp))
    return in_maps


LAST_RESULT = None


def kernel(**inputs) -> np.ndarray:
    global LAST_RESULT
    in_maps = _host_prep(**inputs)
    nc = _build()
    import os
    trace = bool(int(os.environ.get("KERNEL_TRACE", "0")))
    res = run_bass_kernel_spmd(nc, in_maps, core_ids=list(range(8)),
                               trace=trace)
    LAST_RESULT = res
    out = np.empty((B, N, W), np.float32)
    for b in range(B):
        out[b] = res.results[2 * b]["y"] + res.results[2 * b + 1]["y"]
    return out
